# revision 4
# baseline (speedup 1.0000x reference)
"""Trainium2 Bass kernel for the LSTM decoder (nn_Decoder).

Math (reference):
    u0 = x @ W_u0.T + b_u0
    xi0 = [z, u0, enc]                       # CAT = 64 + 128 + 256 = 448
    h0 = xi0 @ W_h1.T + b_h1 ; c0 = xi0 @ W_h2.T + b_h2
    for t in range(T):
        xi = [z, y_{t-1}, enc]               # y_{-1} = u0, y_t = h_t
        gates = xi @ W_ih.T + h @ W_hh.T + b_ih + b_hh
        i,f,g,o = split(gates); c = sig(f)*c + sig(i)*tanh(g); h = sig(o)*tanh(c)
        y_t = h

Key restructuring:
  * z/enc are time-invariant -> their gate contribution gc = z@Wz.T + enc@We.T + b
    is computed once; per-step matmul is only K=128 (h) after merging
    Wc = W_ih[:, y-cols] + W_hh (valid for t >= 1 where y == h).
  * Layout: hidden (128) on partitions, batch on the free dim. h_t is produced
    directly in the rhs layout the next matmul needs -> zero transposes in loop.
  * Gates reordered [i, f, o, g] so one wide Sigmoid covers 3 PSUM banks and
    [sig_i | sig_f] * [tanh_g | c] is a single wide tensor_tensor multiply.
  * Data parallel over 8 cores (batch 8192 -> 1024/core); weights replicated.
"""

import sys

sys.path.insert(0, "/opt/trn_rl_repo")

import numpy as np

import concourse.bass as bass  # noqa: F401  (bass must import before bacc)
import concourse.mybir as mybir
import concourse.tile as tile
from concourse import bacc
from concourse.bass_utils import run_bass_kernel_spmd

N_CORES = 8
BS, IN, HID, LAT, OUT = 8192, 48, 256, 64, 128
B = BS // N_CORES  # 1024 batch rows per core
CH = 512           # batch chunk (one PSUM bank per gate tile)
NCH = B // CH      # 2 chunks
F32 = mybir.dt.float32
AF = mybir.ActivationFunctionType

# gate reorder: torch order i,f,g,o -> i,f,o,g (sigmoid gates contiguous)
GATE_PERM = np.r_[0:128, 128:256, 384:512, 256:384]

_PROGRAM_CACHE: dict = {}


def build_program(n_steps: int, T_out: int):
    """Emit the Bass/Tile program. Runs `n_steps` LSTM steps, writing step t's
    output to y[t % T_out] (n_steps > T_out is used only for timing runs)."""
    nc = bacc.Bacc("TRN2", target_bir_lowering=False, debug=False)

    def inp(name, shape):
        return nc.dram_tensor(name, shape, F32, kind="ExternalInput")

    d_xT = inp("xT", [IN, B])
    d_zT = inp("zT", [LAT, B])
    d_eT0 = inp("encT0", [128, B])
    d_eT1 = inp("encT1", [128, B])
    d_WcT = inp("WcT", [128, 512])
    d_WihyT = inp("WihyT", [128, 512])
    d_WhhT = inp("WhhT", [128, 512])
    d_WzT = inp("WzT", [LAT, 512])
    d_WeT0 = inp("WeT0", [128, 512])
    d_WeT1 = inp("WeT1", [128, 512])
    d_Wu0T = inp("Wu0T", [IN, 128])
    d_Wh = {}
    for nm in ("Wh1", "Wh2"):
        d_Wh[nm] = [inp(f"{nm}z", [LAT, 128]), inp(f"{nm}y", [128, 128]),
                    inp(f"{nm}e0", [128, 128]), inp(f"{nm}e1", [128, 128])]
    d_bias = inp("bias7", [128, 7])  # cols 0-3 gate biases, 4 b_u0, 5 b_h1, 6 b_h2
    d_id = inp("id128", [128, 128])
    d_y = nc.dram_tensor("y", [T_out, OUT, B], F32, kind="ExternalOutput")
    y_ap = d_y[:]

    with tile.TileContext(nc) as tc:
        with (
            tc.tile_pool(name="const", bufs=1) as cpool,
            tc.tile_pool(name="state", bufs=1) as spool,
            tc.tile_pool(name="psum", bufs=1, space="PSUM") as ppool,
        ):
            # ---- load constants / inputs ----
            def load(dram, shape, tag):
                t = cpool.tile(shape, F32, tag=tag, name=tag)
                nc.sync.dma_start(out=t[:], in_=dram[:])
                return t

            xT = load(d_xT, [IN, B], "xT")
            zT = load(d_zT, [LAT, B], "zT")
            eT0 = load(d_eT0, [128, B], "eT0")
            eT1 = load(d_eT1, [128, B], "eT1")
            WcT = load(d_WcT, [128, 512], "WcT")
            WihyT = load(d_WihyT, [128, 512], "WihyT")
            WhhT = load(d_WhhT, [128, 512], "WhhT")
            WzT = load(d_WzT, [LAT, 512], "WzT")
            WeT0 = load(d_WeT0, [128, 512], "WeT0")
            WeT1 = load(d_WeT1, [128, 512], "WeT1")
            Wu0T = load(d_Wu0T, [IN, 128], "Wu0T")
            Wh = {nm: [load(d, [sh, 128], f"{nm}_{i}")
                       for i, (d, sh) in enumerate(zip(ds, (LAT, 128, 128, 128)))]
                  for nm, ds in d_Wh.items()}
            bias = load(d_bias, [128, 7], "bias")
            idm = load(d_id, [128, 128], "idm")

            # ---- persistent loop state ----
            gc = spool.tile([128, 4 * B], F32, tag="gc", name="gc")     # [gate_tile, batch]
            u0 = spool.tile([128, B], F32, tag="u0", name="u0")
            h0 = spool.tile([128, B], F32, tag="h0", name="h0")
            sig = [spool.tile([128, 3 * CH], F32, tag=f"sig{c}", name=f"sig{c}") for c in range(NCH)]
            prod = [spool.tile([128, 2 * CH], F32, tag=f"prod{c}", name=f"prod{c}") for c in range(NCH)]
            tcell = [spool.tile([128, CH], F32, tag=f"tc{c}", name=f"tc{c}") for c in range(NCH)]
            # pair[c][p] = [tanh_g | c_cell] ; h ping-pong per chunk
            pair = [[spool.tile([128, 2 * CH], F32, tag=f"pair{c}{p}", name=f"pair{c}{p}") for p in range(2)]
                    for c in range(NCH)]
            hbuf = [[spool.tile([128, CH], F32, tag=f"h{c}{p}", name=f"h{c}{p}") for p in range(2)]
                    for c in range(NCH)]

            ps = [ppool.tile([128, 2048], F32, tag=f"ps{c}", name=f"ps{c}") for c in range(NCH)]

            MM = nc.tensor.matmul

            # ---- precompute: gc = Wz@z + We@enc + b  (per gate tile) ----
            for c in range(NCH):
                cs = slice(c * CH, (c + 1) * CH)
                for g in range(4):
                    gs = slice(g * 128, (g + 1) * 128)
                    pslice = ps[c][:, g * 512:(g + 1) * 512]
                    MM(pslice, WzT[:, gs], zT[:, cs], start=True, stop=False)
                    MM(pslice, WeT0[:, gs], eT0[:, cs], start=False, stop=False)
                    MM(pslice, WeT1[:, gs], eT1[:, cs], start=False, stop=True)
                    nc.scalar.activation(gc[:, g * B + c * CH: g * B + (c + 1) * CH],
                                         pslice, AF.Identity, bias=bias[:, g:g + 1])

            # ---- precompute: u0, h0, c0 ----
            for c in range(NCH):
                cs = slice(c * CH, (c + 1) * CH)
                pslice = ps[c][:, 0:512]
                MM(pslice, Wu0T[:], xT[:, cs], start=True, stop=True)
                nc.scalar.activation(u0[:, cs], pslice, AF.Identity,
                                     bias=bias[:, 4:5])
            for c in range(NCH):
                cs = slice(c * CH, (c + 1) * CH)
                for W, dst, bcol in ((Wh["Wh1"], h0[:, cs], 5),
                                     (Wh["Wh2"], pair[c][0][:, CH:2 * CH], 6)):
                    pslice = ps[c][:, 512:1024] if bcol == 5 else ps[c][:, 1024:1536]
                    MM(pslice, W[0][:], zT[:, cs], start=True, stop=False)
                    MM(pslice, W[1][:], u0[:, cs], start=False, stop=False)
                    MM(pslice, W[2][:], eT0[:, cs], start=False, stop=False)
                    MM(pslice, W[3][:], eT1[:, cs], start=False, stop=True)
                    nc.scalar.activation(dst, pslice, AF.Identity,
                                         bias=bias[:, bcol:bcol + 1])

            # ---- the scan ----
            for t in range(n_steps):
                par = t % 2
                for c in range(NCH):
                    cs = slice(c * CH, (c + 1) * CH)
                    p = ps[c]
                    for g in range(4):
                        gsl = p[:, g * 512:(g + 1) * 512]
                        wsl = slice(g * 128, (g + 1) * 128)
                        MM(gsl, idm[:], gc[:, g * B + c * CH: g * B + (c + 1) * CH],
                           start=True, stop=False)
                        if t == 0:
                            MM(gsl, WihyT[:, wsl], u0[:, cs], start=False, stop=False)
                            MM(gsl, WhhT[:, wsl], h0[:, cs], start=False, stop=True)
                        else:
                            MM(gsl, WcT[:, wsl], hbuf[c][(t - 1) % 2][:],
                               start=False, stop=True)
                    # pointwise LSTM cell
                    nc.scalar.activation(sig[c][:], p[:, 0:1536], AF.Sigmoid)
                    nc.scalar.activation(pair[c][par][:, 0:CH], p[:, 1536:2048],
                                         AF.Tanh)
                    nc.vector.tensor_mul(out=prod[c][:], in0=sig[c][:, 0:1024],
                                         in1=pair[c][par][:])
                    nc.vector.tensor_add(out=pair[c][1 - par][:, CH:2 * CH],
                                         in0=prod[c][:, 0:CH],
                                         in1=prod[c][:, CH:2 * CH])
                    nc.scalar.activation(tcell[c][:], pair[c][1 - par][:, CH:2 * CH],
                                         AF.Tanh)
                    nc.vector.tensor_mul(out=hbuf[c][par][:],
                                         in0=sig[c][:, 1024:1536], in1=tcell[c][:])
                    nc.sync.dma_start(out=y_ap[t % T_out, :, cs],
                                      in_=hbuf[c][par][:])

    nc.finalize()
    return nc


def _prep_maps(x, enc, z, W_ih, W_hh, b_ih, b_hh, W_u0, b_u0, W_h1, b_h1, W_h2,
               b_h2):
    """Host-side weight prep + per-core sharding. Returns list of in_maps."""
    f = lambda a: np.ascontiguousarray(a, dtype=np.float32)
    p = GATE_PERM
    Wc = (W_ih[:, LAT:LAT + OUT] + W_hh)[p]          # [512, 128]
    bias_g = (b_ih + b_hh)[p].reshape(4, 128).T      # [128, 4]
    bias7 = np.concatenate(
        [bias_g, b_u0[:, None], b_h1[:, None], b_h2[:, None]], axis=1)

    common = {
        "WcT": f(Wc.T),
        "WihyT": f(W_ih[p, LAT:LAT + OUT].T),
        "WhhT": f(W_hh[p].T),
        "WzT": f(W_ih[p, 0:LAT].T),
        "WeT0": f(W_ih[p, LAT + OUT:LAT + OUT + 128].T),
        "WeT1": f(W_ih[p, LAT + OUT + 128:].T),
        "Wu0T": f(W_u0.T),
        "Wh1z": f(W_h1[:, 0:LAT].T), "Wh1y": f(W_h1[:, LAT:LAT + OUT].T),
        "Wh1e0": f(W_h1[:, LAT + OUT:LAT + OUT + 128].T),
        "Wh1e1": f(W_h1[:, LAT + OUT + 128:].T),
        "Wh2z": f(W_h2[:, 0:LAT].T), "Wh2y": f(W_h2[:, LAT:LAT + OUT].T),
        "Wh2e0": f(W_h2[:, LAT + OUT:LAT + OUT + 128].T),
        "Wh2e1": f(W_h2[:, LAT + OUT + 128:].T),
        "bias7": f(bias7),
        "id128": f(np.eye(128)),
    }
    maps = []
    for core in range(N_CORES):
        rows = slice(core * B, (core + 1) * B)
        maps.append({
            "xT": f(x[rows].T), "zT": f(z[rows].T),
            "encT0": f(enc[rows, 0:128].T), "encT1": f(enc[rows, 128:256].T),
            **common,
        })
    return maps


def run_device(maps, n_steps, T_out):
    key = (n_steps, T_out)
    if key not in _PROGRAM_CACHE:
        _PROGRAM_CACHE[key] = build_program(n_steps, T_out)
    nc = _PROGRAM_CACHE[key]
    return run_bass_kernel_spmd(nc, maps, core_ids=list(range(N_CORES)))


def kernel(x, enc, z, W_ih, W_hh, b_ih, b_hh, W_u0, b_u0, W_h1, b_h1, W_h2, b_h2,
           horizon):
    T = int(horizon)
    maps = _prep_maps(np.asarray(x, np.float32), np.asarray(enc, np.float32),
                      np.asarray(z, np.float32), np.asarray(W_ih, np.float32),
                      np.asarray(W_hh, np.float32), np.asarray(b_ih, np.float32),
                      np.asarray(b_hh, np.float32), np.asarray(W_u0, np.float32),
                      np.asarray(b_u0, np.float32), np.asarray(W_h1, np.float32),
                      np.asarray(b_h1, np.float32), np.asarray(W_h2, np.float32),
                      np.asarray(b_h2, np.float32))
    res = run_device(maps, T, T)
    # device y: [T, OUT, B] per core -> [B, T, 1, OUT], concat over cores
    parts = [r["y"].transpose(2, 0, 1)[:, :, None, :] for r in res.results]
    return np.ascontiguousarray(np.concatenate(parts, axis=0), dtype=np.float32)
